# revision 3
# baseline (speedup 1.0000x reference)
"""Trainium2 Bass kernel for LocalGlobalEnvEncoder (GCN + MHA fusion), v3.

Sharding: nodes split across 8 cores (1024 dest nodes / queries each).
 - K/V are computed shard-wise (own 1024 nodes) and exchanged with one bf16
   AllGather; degrees ride a second small AllGather. Degree one-hot matmuls
   run on the PE while the K/V collective is in flight.
 - GCN: edges deduped + bucketed by dest node-tile on host (multiplicity is
   folded into the one-hot scatter matrices); messages gathered from a
   device-materialized y = x * rsqrt(d) table (bf16) by indirect DMA and
   scatter-added with one-hot matmuls on the PE, interleaved with attention.
 - MHA: query-sharded flash-style attention in bf16; scores kept transposed
   ([key, query]) so softmax denominators come from a ones-column in V.
All heavy matmuls run in bf16 with fp32 PSUM accumulation; LayerNorm and
degree math stay fp32. Host only re-lays-out / casts inputs.
"""
import sys
sys.path.insert(0, '/opt/trn_rl_repo')
import numpy as np
import ml_dtypes
import concourse.bass as bass
import concourse.tile as tile
from concourse import bacc, mybir
from concourse.bass_utils import run_bass_kernel_spmd

F32 = mybir.dt.float32
BF16 = mybir.dt.bfloat16
I32 = mybir.dt.int32
AF = mybir.ActivationFunctionType
OP = mybir.AluOpType
AX = mybir.AxisListType

N, E, C, OUTC, H, DH = 8192, 262144, 256, 256, 4, 64
NCORES = 8
NPC = N // NCORES          # nodes per core = 1024
P = 128
NT_LOC = NPC // P          # node tiles per core = 8
NT_GLOB = N // P           # global node tiles = 64
EXP_BIAS = -12.0           # uniform shift inside softmax exp; cancels in the ratio
KVR = 3 * P                # kv allgather rows per core (256 K + 128 V)
KVW = H * (DH + 1) * NT_LOC  # 2080 cols (V row width; K rows use first 1024)

LAST_RESULTS = None


def _build(TPT):
    nc = bacc.Bacc('TRN2', target_bir_lowering=False, debug=False, num_devices=NCORES)
    TE = NT_LOC * TPT

    # ---- I/O ----
    xT_own = nc.dram_tensor("xT_own", [C, NPC], BF16, kind="ExternalInput")
    x_full = nc.dram_tensor("x_full", [N, C], BF16, kind="ExternalInput")
    x_own = nc.dram_tensor("x_own", [NPC, C], F32, kind="ExternalInput")
    WqT = nc.dram_tensor("WqT", [C, C], BF16, kind="ExternalInput")
    WkT = nc.dram_tensor("WkT", [C, C], BF16, kind="ExternalInput")
    WvT = nc.dram_tensor("WvT", [C, C], BF16, kind="ExternalInput")
    WopT = nc.dram_tensor("WopT", [C, C], BF16, kind="ExternalInput")
    Wl = nc.dram_tensor("Wl", [C, C], BF16, kind="ExternalInput")
    fcT = nc.dram_tensor("fcT", [C, OUTC], BF16, kind="ExternalInput")
    bq_pack = nc.dram_tensor("bq_pack", [P, 2], F32, kind="ExternalInput")
    bk_pack = nc.dram_tensor("bk_pack", [P, 2], F32, kind="ExternalInput")
    bv_rep = nc.dram_tensor("bv_rep", [P, C], F32, kind="ExternalInput")
    opb_rep = nc.dram_tensor("opb_rep", [P, C], F32, kind="ExternalInput")
    g_rep = nc.dram_tensor("g_rep", [P, C], F32, kind="ExternalInput")
    b_rep = nc.dram_tensor("b_rep", [P, C], F32, kind="ExternalInput")
    fcb_rep = nc.dram_tensor("fcb_rep", [P, OUTC], F32, kind="ExternalInput")
    alpha11 = nc.dram_tensor("alpha11", [1, 1], F32, kind="ExternalInput")
    iota_in = nc.dram_tensor("iota_in", [P, P], F32, kind="ExternalInput")
    ident_in = nc.dram_tensor("ident_in", [P, P], F32, kind="ExternalInput")
    ones_col_in = nc.dram_tensor("ones_col_in", [P, 1], F32, kind="ExternalInput")
    ones_row_in = nc.dram_tensor("ones_row_in", [1, P], F32, kind="ExternalInput")
    col_adj = nc.dram_tensor("col_adj", [P, TE], I32, kind="ExternalInput")
    row_idx = nc.dram_tensor("row_idx", [P, TE], I32, kind="ExternalInput")
    mult_in = nc.dram_tensor("mult_in", [P, TE], F32, kind="ExternalInput")

    out = nc.dram_tensor("out", [NPC, OUTC], F32, kind="ExternalOutput")
    y_scr = nc.dram_tensor("y_scr", [N, C], BF16, kind="ExternalOutput")  # scratch

    with tile.TileContext(nc) as tc:
        with tc.tile_pool(name="const", bufs=1) as const, \
             tc.tile_pool(name="big", bufs=1) as big, \
             tc.tile_pool(name="dram", bufs=1, space="DRAM") as dram:

            # ---- persistent constants ----
            iota_t = const.tile([P, P], F32)
            nc.sync.dma_start(out=iota_t[:], in_=iota_in[:])
            iota_b = const.tile([P, P], BF16)
            nc.vector.tensor_copy(out=iota_b[:], in_=iota_t[:])
            ident_b = const.tile([P, P], BF16)
            nc.vector.tensor_copy(out=ident_b[:], in_=iota_t[:])  # placeholder; overwritten below
            identf = const.tile([P, P], F32)
            nc.sync.dma_start(out=identf[:], in_=ident_in[:])
            nc.vector.tensor_copy(out=ident_b[:], in_=identf[:])
            ones_col_t = const.tile([P, 1], F32)
            nc.sync.dma_start(out=ones_col_t[:], in_=ones_col_in[:])
            ones_col_b = const.tile([P, 1], BF16)
            nc.vector.tensor_copy(out=ones_col_b[:], in_=ones_col_t[:])
            ones_row_t = const.tile([1, P], F32)
            nc.sync.dma_start(out=ones_row_t[:], in_=ones_row_in[:])
            col_t = const.tile([P, TE], I32)
            nc.sync.dma_start(out=col_t[:], in_=col_adj[:])
            row_t = const.tile([P, TE], I32)
            nc.sync.dma_start(out=row_t[:], in_=row_idx[:])
            colf_t = const.tile([P, TE], F32)
            nc.vector.tensor_copy(out=colf_t[:], in_=col_t[:])
            mult_t = const.tile([P, TE], F32)
            nc.sync.dma_start(out=mult_t[:], in_=mult_in[:])
            expb_col = const.tile([P, 1], F32)
            nc.vector.memset(expb_col[:], EXP_BIAS)
            eps_col = const.tile([P, 1], F32)
            nc.vector.memset(eps_col[:], 1e-5)

            d_loc = const.tile([P, NT_LOC], F32)
            s_own = const.tile([P, NT_LOC], F32)
            s_all = const.tile([P, NT_GLOB], F32)
            w_col = const.tile([P, 1], F32)

            # ---- persistent big tiles ----
            KTp = [big.tile([P, N], BF16, name=f"KT{p}") for p in range(2)]
            QTp = [big.tile([P, NPC], BF16, name=f"QT{p}") for p in range(2)]
            Vt = big.tile([P, NT_GLOB * H * (DH + 1)], BF16, name="Vt")
            V4 = Vt[:].rearrange("p (k h d) -> p k h d", h=H, d=DH + 1)
            k_sb = [big.tile([P, NPC], BF16, name=f"ksb{p}") for p in range(2)]
            v_sb = big.tile([P, KVW], BF16, name="vsb")
            V4own = v_sb[:].rearrange("p (k h d) -> p k h d", h=H, d=DH + 1)
            O_all = [big.tile([P, C], BF16, name=f"Oall{i}") for i in range(NT_LOC)]
            hi_sb = [big.tile([P, C], BF16, name=f"hi{i}") for i in range(NT_LOC)]

            kv_in = dram.tile([KVR, KVW], BF16)
            kv_out = dram.tile([NCORES * KVR, KVW], BF16)
            dg_in = dram.tile([NT_LOC, P], F32)
            dg_out = dram.tile([NT_GLOB, P], F32)

            # ============ phase A: own-shard K/V projections + KV AllGather ======
            with tc.tile_pool(name="phA", bufs=1) as phA, \
                 tc.tile_pool(name="psA", bufs=1, space="PSUM") as psA:
                Wq_t = phA.tile([P, 2 * C], BF16)
                nc.sync.dma_start(out=Wq_t[:].rearrange("p (c n) -> p c n", c=2), in_=WqT[:].rearrange("(c p) n -> p c n", p=P))
                Wk_t = phA.tile([P, 2 * C], BF16)
                nc.sync.dma_start(out=Wk_t[:].rearrange("p (c n) -> p c n", c=2), in_=WkT[:].rearrange("(c p) n -> p c n", p=P))
                Wv_t = phA.tile([P, 2 * C], BF16)
                nc.sync.dma_start(out=Wv_t[:].rearrange("p (c n) -> p c n", c=2), in_=WvT[:].rearrange("(c p) n -> p c n", p=P))
                bq_t = phA.tile([P, 2], F32)
                nc.sync.dma_start(out=bq_t[:], in_=bq_pack[:])
                bk_t = phA.tile([P, 2], F32)
                nc.sync.dma_start(out=bk_t[:], in_=bk_pack[:])
                bv_t = phA.tile([P, C], F32)
                nc.sync.dma_start(out=bv_t[:], in_=bv_rep[:])
                xo = [phA.tile([P, NPC], BF16, name=f"xo{c}") for c in range(2)]
                for c in range(2):
                    nc.sync.dma_start(out=xo[c][:], in_=xT_own[c * P:(c + 1) * P, :])

                # K own
                for p in range(2):
                    for nb in range(NPC // 512):
                        kps = psA.tile([P, 512], F32, tag="qkps", bufs=2)
                        for c in range(2):
                            nc.tensor.matmul(
                                out=kps[:],
                                lhsT=Wk_t[:, c * C + p * P: c * C + (p + 1) * P],
                                rhs=xo[c][:, nb * 512:(nb + 1) * 512],
                                start=(c == 0), stop=(c == 1))
                        nc.vector.tensor_scalar(
                            out=k_sb[p][:, nb * 512:(nb + 1) * 512], in0=kps[:],
                            scalar1=bk_t[:, p:p + 1], scalar2=None, op0=OP.add)
                # V own (ones col for denominators)
                nc.vector.memset(V4own[:, :, :, DH:DH + 1], 1.0)
                for ntl in range(NT_LOC):
                    vps = psA.tile([P, C], F32, tag="vps", bufs=2)
                    for c in range(2):
                        nc.tensor.matmul(
                            out=vps[:],
                            lhsT=xo[c][:, ntl * P:(ntl + 1) * P],
                            rhs=Wv_t[:, c * C:(c + 1) * C],
                            start=(c == 0), stop=(c == 1))
                    nc.vector.tensor_tensor(
                        out=V4own[:, ntl, :, 0:DH],
                        in0=vps[:].rearrange("p (h d) -> p h d", d=DH),
                        in1=bv_t[:].rearrange("p (h d) -> p h d", d=DH),
                        op=OP.add)
                # stage K/V and launch the big AllGather
                for p in range(2):
                    nc.scalar.dma_start(out=kv_in[p * P:(p + 1) * P, 0:NPC], in_=k_sb[p][:])
                nc.scalar.dma_start(out=kv_in[2 * P:3 * P, :], in_=v_sb[:])
                nc.gpsimd.collective_compute(
                    "AllGather", OP.bypass,
                    replica_groups=[list(range(NCORES))],
                    ins=[kv_in[:].opt()], outs=[kv_out[:].opt()])

                # ---- degrees on PE while the collective runs ----
                al_t = phA.tile([1, 1], F32)
                nc.scalar.dma_start(out=al_t[:], in_=alpha11[:])
                wsig = phA.tile([1, 1], F32)
                nc.scalar.activation(out=wsig[:], in_=al_t[:], func=AF.Sigmoid)
                wrep_ps = psA.tile([P, 1], F32, tag="wrep")
                nc.tensor.matmul(out=wrep_ps[:], lhsT=ones_row_t[:], rhs=wsig[:],
                                 start=True, stop=True)
                nc.vector.tensor_copy(out=w_col[:], in_=wrep_ps[:])

                for t in range(NT_LOC):
                    dps = psA.tile([P, 1], F32, tag="dps", bufs=2)
                    for i in range(TPT):
                        j = t * TPT + i
                        oh = phA.tile([P, P], BF16, tag="oh", bufs=3)
                        nc.vector.tensor_scalar(
                            out=oh[:], in0=iota_b[:], scalar1=colf_t[:, j:j + 1],
                            scalar2=mult_t[:, j:j + 1], op0=OP.is_equal, op1=OP.mult)
                        nc.tensor.matmul(out=dps[:], lhsT=oh[:], rhs=ones_col_b[:],
                                         start=(i == 0), stop=(i == TPT - 1))
                    nc.vector.tensor_copy(out=d_loc[:, t:t + 1], in_=dps[:])
                    nc.scalar.dma_start(out=dg_in[t, :, None], in_=d_loc[:, t:t + 1])
                nc.gpsimd.collective_compute(
                    "AllGather", OP.bypass,
                    replica_groups=[list(range(NCORES))],
                    ins=[dg_in[:].opt()], outs=[dg_out[:].opt()])

                # Q projection (fills PE while collectives run)
                for p in range(2):
                    for nb in range(NPC // 512):
                        qps = psA.tile([P, 512], F32, tag="qkps", bufs=2)
                        for c in range(2):
                            nc.tensor.matmul(
                                out=qps[:],
                                lhsT=Wq_t[:, c * C + p * P: c * C + (p + 1) * P],
                                rhs=xo[c][:, nb * 512:(nb + 1) * 512],
                                start=(c == 0), stop=(c == 1))
                        nc.vector.tensor_scalar(
                            out=QTp[p][:, nb * 512:(nb + 1) * 512], in0=qps[:],
                            scalar1=bq_t[:, p:p + 1], scalar2=None, op0=OP.add)

            # ============ phase B: gather K/V, rsqrt(d), y table =================
            # K/V loads on the sync queue (gates attention start)
            for k in range(NCORES):
                for p in range(2):
                    nc.sync.dma_start(
                        out=KTp[p][:, k * NPC:(k + 1) * NPC],
                        in_=kv_out[k * KVR + p * P: k * KVR + (p + 1) * P, 0:NPC])
            for k in range(NCORES):
                nc.sync.dma_start(
                    out=Vt[:, k * KVW:(k + 1) * KVW],
                    in_=kv_out[k * KVR + 2 * P: (k + 1) * KVR, :])

            with tc.tile_pool(name="phB", bufs=1) as phB:
                d_all = phB.tile([P, NT_GLOB], F32)
                nc.scalar.dma_start(out=d_all[:], in_=dg_out[:].rearrange("g p -> p g"))
                for (src, dst, w_) in ((d_all, s_all, NT_GLOB), (d_loc, s_own, NT_LOC)):
                    m_t = phB.tile([P, w_], F32, tag=f"m{w_}")
                    nc.vector.tensor_scalar(out=m_t[:], in0=src[:], scalar1=1.0,
                                            scalar2=None, op0=OP.min)
                    t1 = phB.tile([P, w_], F32, tag=f"t1{w_}")
                    nc.vector.tensor_scalar(out=t1[:], in0=src[:], scalar1=1.0,
                                            scalar2=None, op0=OP.add)
                    nc.vector.tensor_tensor(out=t1[:], in0=t1[:], in1=m_t[:],
                                            op=OP.subtract)
                    nc.scalar.activation(out=t1[:], in_=t1[:], func=AF.Sqrt)
                    nc.vector.reciprocal(out=t1[:], in_=t1[:])
                    nc.vector.tensor_tensor(out=dst[:], in0=t1[:], in1=m_t[:],
                                            op=OP.mult)
                # y = x * rsqrt(d)[node] -> DRAM (bf16)
                for g in range(NT_GLOB):
                    xt = phB.tile([P, C], BF16, tag="xt", bufs=3)
                    nc.scalar.dma_start(out=xt[:], in_=x_full[g * P:(g + 1) * P, :])
                    yt = phB.tile([P, C], BF16, tag="yt", bufs=3)
                    nc.vector.tensor_scalar(out=yt[:], in0=xt[:],
                                            scalar1=s_all[:, g:g + 1],
                                            scalar2=None, op0=OP.mult)
                    nc.scalar.dma_start(out=y_scr[g * P:(g + 1) * P, :], in_=yt[:])

            # ============ phase C: attention + interleaved GCN scatter ===========
            with tc.tile_pool(name="phC", bufs=1) as phC, \
                 tc.tile_pool(name="psC", bufs=1, space="PSUM") as psC:

                scat_jobs = [(t, i) for t in range(NT_LOC) for i in range(TPT)]
                n_jobs = len(scat_jobs)
                n_steps = H * NT_GLOB
                START_STEP = 24          # let the y table land first
                emitted = 0
                hips_cur = {}

                def emit_scatter_jobs(upto):
                    nonlocal emitted
                    while emitted < min(upto, n_jobs):
                        t, i = scat_jobs[emitted]
                        j = t * TPT + i
                        if i == 0:
                            hips_cur[t] = psC.tile([P, C], F32, tag="hips", bufs=2, name=f"hips{t}")
                        yg = phC.tile([P, C], BF16, tag="yg", bufs=6)
                        nc.gpsimd.indirect_dma_start(
                            out=yg[:], out_offset=None, in_=y_scr[:],
                            in_offset=bass.IndirectOffsetOnAxis(
                                ap=row_t[:, j:j + 1], axis=0))
                        oh = phC.tile([P, P], BF16, tag="oh2", bufs=3)
                        nc.vector.tensor_scalar(
                            out=oh[:], in0=iota_b[:], scalar1=colf_t[:, j:j + 1],
                            scalar2=mult_t[:, j:j + 1], op0=OP.is_equal, op1=OP.mult)
                        nc.tensor.matmul(out=hips_cur[t][:], lhsT=oh[:], rhs=yg[:],
                                         start=(i == 0), stop=(i == TPT - 1))
                        if i == TPT - 1:
                            nc.vector.tensor_scalar(out=hi_sb[t][:], in0=hips_cur[t][:],
                                                    scalar1=s_own[:, t:t + 1],
                                                    scalar2=None, op0=OP.mult)
                        emitted += 1

                for h in range(H):
                    p, hh = h // 2, h % 2
                    po = hh * DH
                    Ops = [psC.tile([P, 4 * (DH + 1)], F32, tag=f"O{i}", bufs=1, name=f"O{i}h{h}")
                           for i in range(2)]
                    for kt in range(NT_GLOB):
                        sps = psC.tile([P, NPC], F32, tag="sps", bufs=2)
                        for qh in range(2):
                            nc.tensor.matmul(
                                out=sps[:, qh * 512:(qh + 1) * 512],
                                lhsT=KTp[p][po:po + DH, kt * P:(kt + 1) * P],
                                rhs=QTp[p][po:po + DH, qh * 512:(qh + 1) * 512],
                                start=True, stop=True)
                        et = phC.tile([P, NPC], BF16, tag="expT", bufs=2)
                        nc.scalar.activation(out=et[:], in_=sps[:], func=AF.Exp,
                                             bias=expb_col[:, 0:1], scale=1.0 / np.sqrt(DH))
                        for qt in range(NT_LOC):
                            nc.tensor.matmul(
                                out=Ops[qt // 4][:, (qt % 4) * (DH + 1):(qt % 4 + 1) * (DH + 1)],
                                lhsT=et[:, qt * P:(qt + 1) * P],
                                rhs=V4[:, kt, h, :],
                                start=(kt == 0), stop=(kt == NT_GLOB - 1))
                        step = h * NT_GLOB + kt + 1
                        if step > START_STEP:
                            emit_scatter_jobs(n_jobs * (step - START_STEP) // (n_steps - START_STEP))
                    for qt in range(NT_LOC):
                        sl0 = (qt % 4) * (DH + 1)
                        den = phC.tile([P, 1], F32, tag="den", bufs=2)
                        nc.vector.reciprocal(out=den[:],
                                             in_=Ops[qt // 4][:, sl0 + DH:sl0 + DH + 1])
                        nc.vector.tensor_scalar(
                            out=O_all[qt][:, h * DH:(h + 1) * DH],
                            in0=Ops[qt // 4][:, sl0:sl0 + DH],
                            scalar1=den[:, 0:1], scalar2=None, op0=OP.mult)

            # ============ phase D: out_proj, LN, combine, fc (bf16 matmuls) ======
            with tc.tile_pool(name="phD", bufs=1) as phD, \
                 tc.tile_pool(name="psD", bufs=1, space="PSUM") as psD:
                Wop_t = phD.tile([P, 2 * C], BF16)
                nc.sync.dma_start(out=Wop_t[:].rearrange("p (c n) -> p c n", c=2), in_=WopT[:].rearrange("(c p) n -> p c n", p=P))
                Wl_t = phD.tile([P, 2 * C], BF16)
                nc.sync.dma_start(out=Wl_t[:].rearrange("p (c n) -> p c n", c=2), in_=Wl[:].rearrange("(c p) n -> p c n", p=P))
                fc_t = phD.tile([P, 2 * OUTC], BF16)
                nc.sync.dma_start(out=fc_t[:].rearrange("p (c n) -> p c n", c=2), in_=fcT[:].rearrange("(c p) n -> p c n", p=P))
                opb_t = phD.tile([P, C], F32)
                nc.sync.dma_start(out=opb_t[:], in_=opb_rep[:])
                g_t = phD.tile([P, C], F32)
                nc.sync.dma_start(out=g_t[:], in_=g_rep[:])
                b_t = phD.tile([P, C], F32)
                nc.sync.dma_start(out=b_t[:], in_=b_rep[:])
                fcb_t = phD.tile([P, OUTC], F32)
                nc.sync.dma_start(out=fcb_t[:], in_=fcb_rep[:])

                def transpose_2chunks(src_ap, tag):
                    dst = phD.tile([P, C], BF16, tag=tag, bufs=2)
                    for c in range(2):
                        tp = psD.tile([P, P], BF16, tag="tp", bufs=2)
                        nc.tensor.transpose(out=tp[:], in_=src_ap[:, c * P:(c + 1) * P],
                                            identity=ident_b[:])
                        nc.vector.tensor_copy(out=dst[:, c * P:(c + 1) * P], in_=tp[:])
                    return dst

                for qt in range(NT_LOC):
                    # ---- global path: out_proj + residual + LN ----
                    OT = transpose_2chunks(O_all[qt][:], "OT")
                    aps = psD.tile([P, C], F32, tag="aps", bufs=2)
                    for c in range(2):
                        nc.tensor.matmul(out=aps[:], lhsT=OT[:, c * P:(c + 1) * P],
                                         rhs=Wop_t[:, c * C:(c + 1) * C],
                                         start=(c == 0), stop=(c == 1))
                    v_t = phD.tile([P, C], F32, tag="vt", bufs=2)
                    nc.vector.tensor_tensor(out=v_t[:], in0=aps[:], in1=opb_t[:], op=OP.add)
                    xo_t = phD.tile([P, C], F32, tag="xot", bufs=2)
                    nc.sync.dma_start(out=xo_t[:], in_=x_own[qt * P:(qt + 1) * P, :])
                    nc.vector.tensor_tensor(out=v_t[:], in0=v_t[:], in1=xo_t[:], op=OP.add)
                    # mean via ACT accumulate
                    vcop = phD.tile([P, C], F32, tag="vcop", bufs=2)
                    msum = phD.tile([P, 1], F32, tag="msum", bufs=2)
                    nc.scalar.activation(out=vcop[:], in_=v_t[:], func=AF.Copy,
                                         accum_out=msum[:])
                    mean = phD.tile([P, 1], F32, tag="mean", bufs=2)
                    nc.vector.tensor_scalar(out=mean[:], in0=msum[:], scalar1=1.0 / C,
                                            scalar2=None, op0=OP.mult)
                    nc.vector.tensor_scalar(out=v_t[:], in0=v_t[:], scalar1=mean[:, 0:1],
                                            scalar2=None, op0=OP.subtract)
                    # var via ACT Square accumulate
                    sq = phD.tile([P, C], F32, tag="sq", bufs=2)
                    ssum = phD.tile([P, 1], F32, tag="ssum", bufs=2)
                    nc.scalar.activation(out=sq[:], in_=v_t[:], func=AF.Square,
                                         accum_out=ssum[:])
                    sstd = phD.tile([P, 1], F32, tag="sstd", bufs=2)
                    nc.scalar.activation(out=sstd[:], in_=ssum[:], func=AF.Sqrt,
                                         bias=eps_col[:, 0:1], scale=1.0 / C)
                    rstd = phD.tile([P, 1], F32, tag="rstd", bufs=2)
                    nc.vector.reciprocal(out=rstd[:], in_=sstd[:])
                    vn = phD.tile([P, C], F32, tag="vn", bufs=2)
                    nc.scalar.activation(out=vn[:], in_=v_t[:], func=AF.Copy,
                                         scale=rstd[:, 0:1])
                    nc.vector.tensor_tensor(out=vn[:], in0=vn[:], in1=g_t[:], op=OP.mult)
                    nc.vector.tensor_tensor(out=vn[:], in0=vn[:], in1=b_t[:], op=OP.add)

                    # ---- local path ----
                    hiT = transpose_2chunks(hi_sb[qt][:], "hiT")
                    lps = psD.tile([P, C], F32, tag="lps", bufs=2)
                    for c in range(2):
                        nc.tensor.matmul(out=lps[:], lhsT=hiT[:, c * P:(c + 1) * P],
                                         rhs=Wl_t[:, c * C:(c + 1) * C],
                                         start=(c == 0), stop=(c == 1))
                    # combined = global + w * (local - global), emitted in bf16
                    df = phD.tile([P, C], F32, tag="df", bufs=2)
                    nc.vector.tensor_tensor(out=df[:], in0=lps[:], in1=vn[:],
                                            op=OP.subtract)
                    nc.vector.tensor_scalar(out=df[:], in0=df[:], scalar1=w_col[:, 0:1],
                                            scalar2=None, op0=OP.mult)
                    comb = phD.tile([P, C], BF16, tag="comb", bufs=2)
                    nc.vector.tensor_tensor(out=comb[:], in0=df[:], in1=vn[:], op=OP.add)

                    # ---- fc ----
                    cT = transpose_2chunks(comb[:], "cT")
                    fps = psD.tile([P, OUTC], F32, tag="fps", bufs=2)
                    for c in range(2):
                        nc.tensor.matmul(out=fps[:], lhsT=cT[:, c * P:(c + 1) * P],
                                         rhs=fc_t[:, c * OUTC:(c + 1) * OUTC],
                                         start=(c == 0), stop=(c == 1))
                    o_t = phD.tile([P, OUTC], F32, tag="ot", bufs=2)
                    nc.vector.tensor_tensor(out=o_t[:], in0=fps[:], in1=fcb_t[:], op=OP.add)
                    nc.sync.dma_start(out=out[qt * P:(qt + 1) * P, :], in_=o_t[:])
    nc.finalize()
    return nc


def _prep_edges(adj):
    """Dedup (row, col) pairs, bucket by destination node-tile, pad segments.

    Returns per-core (col_adj[P, TE], row_idx[P, TE], mult[P, TE]) and TPT.
    """
    row = np.asarray(adj[0], dtype=np.int64)
    col = np.asarray(adj[1], dtype=np.int64)
    key = col * N + row
    ukey, counts = np.unique(key, return_counts=True)
    ucol = (ukey // N).astype(np.int64)
    urow = (ukey % N).astype(np.int64)
    tid = ucol // P
    # unique keys are already sorted by col, hence by tid
    seg_counts = np.bincount(tid, minlength=NT_GLOB)
    S = int(np.ceil(max(seg_counts.max(), 1) / P) * P)
    TPT = S // P
    col_pad = np.full((NT_GLOB, S), -1, dtype=np.int32)
    row_pad = np.zeros((NT_GLOB, S), dtype=np.int32)
    mul_pad = np.zeros((NT_GLOB, S), dtype=np.float32)
    start = 0
    for g in range(NT_GLOB):
        cnt = int(seg_counts[g])
        col_pad[g, :cnt] = (ucol[start:start + cnt] - g * P).astype(np.int32)
        row_pad[g, :cnt] = urow[start:start + cnt].astype(np.int32)
        mul_pad[g, :cnt] = counts[start:start + cnt].astype(np.float32)
        start += cnt
    col_pad = col_pad.reshape(NT_GLOB, TPT, P)
    row_pad = row_pad.reshape(NT_GLOB, TPT, P)
    mul_pad = mul_pad.reshape(NT_GLOB, TPT, P)
    per_core = []
    for k in range(NCORES):
        ca = col_pad[NT_LOC * k:NT_LOC * (k + 1)].reshape(NT_LOC * TPT, P).T
        ri = row_pad[NT_LOC * k:NT_LOC * (k + 1)].reshape(NT_LOC * TPT, P).T
        mu = mul_pad[NT_LOC * k:NT_LOC * (k + 1)].reshape(NT_LOC * TPT, P).T
        per_core.append((np.ascontiguousarray(ca), np.ascontiguousarray(ri),
                         np.ascontiguousarray(mu)))
    return per_core, TPT


def kernel(x, adj, weight_local, in_proj_w, in_proj_b, out_proj_w, out_proj_b,
           ln_g, ln_b, alpha, fc_w, fc_b):
    global LAST_RESULTS
    BF = ml_dtypes.bfloat16
    x = np.ascontiguousarray(np.asarray(x, dtype=np.float32))
    per_core_edges, TPT = _prep_edges(np.asarray(adj))

    xT_b = np.ascontiguousarray(x.T).astype(BF)
    common = dict(
        x_full=x.astype(BF),
        WqT=np.ascontiguousarray(np.asarray(in_proj_w)[0:C].T.astype(np.float32)).astype(BF),
        WkT=np.ascontiguousarray(np.asarray(in_proj_w)[C:2 * C].T.astype(np.float32)).astype(BF),
        WvT=np.ascontiguousarray(np.asarray(in_proj_w)[2 * C:3 * C].T.astype(np.float32)).astype(BF),
        WopT=np.ascontiguousarray(np.asarray(out_proj_w).T.astype(np.float32)).astype(BF),
        Wl=np.ascontiguousarray(np.asarray(weight_local, dtype=np.float32)).astype(BF),
        fcT=np.ascontiguousarray(np.asarray(fc_w).T.astype(np.float32)).astype(BF),
        bq_pack=np.ascontiguousarray(np.asarray(in_proj_b)[0:C].astype(np.float32).reshape(2, P).T),
        bk_pack=np.ascontiguousarray(np.asarray(in_proj_b)[C:2 * C].astype(np.float32).reshape(2, P).T),
        bv_rep=np.tile(np.asarray(in_proj_b)[2 * C:3 * C].astype(np.float32), (P, 1)),
        opb_rep=np.tile(np.asarray(out_proj_b, dtype=np.float32), (P, 1)),
        g_rep=np.tile(np.asarray(ln_g, dtype=np.float32), (P, 1)),
        b_rep=np.tile(np.asarray(ln_b, dtype=np.float32), (P, 1)),
        fcb_rep=np.tile(np.asarray(fc_b, dtype=np.float32), (P, 1)),
        alpha11=np.asarray(alpha, dtype=np.float32).reshape(1, 1),
        iota_in=np.tile(np.arange(P, dtype=np.float32), (P, 1)),
        ident_in=np.eye(P, dtype=np.float32),
        ones_col_in=np.ones((P, 1), dtype=np.float32),
        ones_row_in=np.ones((1, P), dtype=np.float32),
    )
    in_maps = []
    for k in range(NCORES):
        ca, ri, mu = per_core_edges[k]
        m = dict(common)
        m['xT_own'] = np.ascontiguousarray(xT_b[:, k * NPC:(k + 1) * NPC])
        m['x_own'] = np.ascontiguousarray(x[k * NPC:(k + 1) * NPC, :])
        m['col_adj'] = ca
        m['row_idx'] = ri
        m['mult_in'] = mu
        in_maps.append(m)

    nc = _build(TPT)
    res = run_bass_kernel_spmd(nc, in_maps, core_ids=list(range(NCORES)))
    LAST_RESULTS = res
    return np.concatenate([res.results[k]['out'] for k in range(NCORES)], axis=0)


# revision 6
# speedup vs baseline: 1.2171x; 1.2171x over previous
"""Trainium2 Bass kernel for LocalGlobalEnvEncoder (GCN + MHA fusion), v4.

Sharding: nodes split across 8 cores (1024 dest nodes / queries each).
 - K/V are computed shard-wise (own 1024 nodes) and exchanged with one bf16
   AllGather (V rides along as fp8 bytes). Node degrees are an integer
   histogram of the edge list, computed host-side with the rest of the edge
   preprocessing; rsqrt scaling stays on device.
 - GCN: edges bucketed by dest node-tile on host (duplicate edges folded into
   one-hot multiplicities); messages gathered from a device-materialized
   y = x * rsqrt(d) table (bf16) by indirect DMA and scatter-added with
   one-hot matmuls on the PE, interleaved with attention from step 0.
 - MHA: query-sharded flash attention; QK^T in bf16, exp -> fp8 scores, and
   attn@V as fp8 DoubleRow matmuls over kt-pairs (256-deep contraction).
   Scores stay transposed ([key, query]) so softmax denominators come from a
   ones-column in V.
Matmuls accumulate in fp32 PSUM; LayerNorm stays fp32.
"""
import sys
sys.path.insert(0, '/opt/trn_rl_repo')
import numpy as np
import ml_dtypes
import concourse.bass as bass
import concourse.tile as tile
from concourse import bacc, mybir
from concourse.bass_utils import run_bass_kernel_spmd

F32 = mybir.dt.float32
BF16 = mybir.dt.bfloat16
FP8 = mybir.dt.float8e4
FP8E5 = mybir.dt.float8e5
I32 = mybir.dt.int32
AF = mybir.ActivationFunctionType
OP = mybir.AluOpType
AX = mybir.AxisListType
DR = mybir.MatmulPerfMode.DoubleRow

N, E, C, OUTC, H, DH = 8192, 262144, 256, 256, 4, 64
NCORES = 8
NPC = N // NCORES          # 1024 nodes per core
P = 128
NT_LOC = NPC // P          # 8
NT_GLOB = N // P           # 64
EXP_BIAS = 0.0             # e5m2 spans the whole exp(score) range; cancels in softmax
KVR = 3 * P                # allgather rows per core: 256 K rows + 128 V rows
KVW = NPC + 16             # bf16 row width: K uses 1024; V fp8 rows bitcast to 1040

LAST_RESULTS = None


def _build(TPT):
    nc = bacc.Bacc('TRN2', target_bir_lowering=False, debug=False, num_devices=NCORES)
    TE = NT_LOC * TPT

    # ---- I/O ----
    xT_own = nc.dram_tensor("xT_own", [C, NPC], BF16, kind="ExternalInput")
    x_full = nc.dram_tensor("x_full", [N, C], BF16, kind="ExternalInput")
    x_own = nc.dram_tensor("x_own", [NPC, C], F32, kind="ExternalInput")
    WqT = nc.dram_tensor("WqT", [C, C], BF16, kind="ExternalInput")
    WkT = nc.dram_tensor("WkT", [C, C], BF16, kind="ExternalInput")
    WvT = nc.dram_tensor("WvT", [C, C], BF16, kind="ExternalInput")
    WopT = nc.dram_tensor("WopT", [C, C], BF16, kind="ExternalInput")
    Wl = nc.dram_tensor("Wl", [C, C], BF16, kind="ExternalInput")
    fcT = nc.dram_tensor("fcT", [C, OUTC], BF16, kind="ExternalInput")
    bq_pack = nc.dram_tensor("bq_pack", [P, 2], F32, kind="ExternalInput")
    bk_pack = nc.dram_tensor("bk_pack", [P, 2], F32, kind="ExternalInput")
    bv_rep = nc.dram_tensor("bv_rep", [P, C], F32, kind="ExternalInput")
    opb_rep = nc.dram_tensor("opb_rep", [P, C], F32, kind="ExternalInput")
    g_rep = nc.dram_tensor("g_rep", [P, C], F32, kind="ExternalInput")
    b_rep = nc.dram_tensor("b_rep", [P, C], F32, kind="ExternalInput")
    fcb_rep = nc.dram_tensor("fcb_rep", [P, OUTC], F32, kind="ExternalInput")
    alpha11 = nc.dram_tensor("alpha11", [1, 1], F32, kind="ExternalInput")
    iota_in = nc.dram_tensor("iota_in", [P, P], F32, kind="ExternalInput")
    ident_in = nc.dram_tensor("ident_in", [P, P], F32, kind="ExternalInput")
    ones_row_in = nc.dram_tensor("ones_row_in", [1, P], F32, kind="ExternalInput")
    deg_all = nc.dram_tensor("deg_all", [P, NT_GLOB], F32, kind="ExternalInput")
    deg_own = nc.dram_tensor("deg_own", [P, NT_LOC], F32, kind="ExternalInput")
    col_adj = nc.dram_tensor("col_adj", [P, TE], I32, kind="ExternalInput")
    row_idx = nc.dram_tensor("row_idx", [P, TE], I32, kind="ExternalInput")
    mult_in = nc.dram_tensor("mult_in", [P, TE], F32, kind="ExternalInput")

    out = nc.dram_tensor("out", [NPC, OUTC], F32, kind="ExternalOutput")
    y_scr = nc.dram_tensor("y_scr", [N, C], BF16, kind="ExternalOutput")  # scratch

    with tile.TileContext(nc) as tc:
        with tc.tile_pool(name="const", bufs=1) as const, \
             tc.tile_pool(name="big", bufs=1) as big, \
             tc.tile_pool(name="dram", bufs=1, space="DRAM") as dram:

            # ---- persistent constants ----
            iota_t = const.tile([P, P], F32)
            nc.sync.dma_start(out=iota_t[:], in_=iota_in[:])
            iota_b = const.tile([P, P], BF16)
            nc.vector.tensor_copy(out=iota_b[:], in_=iota_t[:])
            identf = const.tile([P, P], F32)
            nc.sync.dma_start(out=identf[:], in_=ident_in[:])
            ident_b = const.tile([P, P], BF16)
            nc.vector.tensor_copy(out=ident_b[:], in_=identf[:])
            ones_row_t = const.tile([1, P], F32)
            nc.sync.dma_start(out=ones_row_t[:], in_=ones_row_in[:])
            col_t = const.tile([P, TE], I32)
            nc.sync.dma_start(out=col_t[:], in_=col_adj[:])
            row_t = const.tile([P, TE], I32)
            nc.sync.dma_start(out=row_t[:], in_=row_idx[:])
            colf_t = const.tile([P, TE], F32)
            nc.vector.tensor_copy(out=colf_t[:], in_=col_t[:])
            mult_t = const.tile([P, TE], F32)
            nc.sync.dma_start(out=mult_t[:], in_=mult_in[:])
            expb_col = const.tile([P, 1], F32)
            nc.vector.memset(expb_col[:], EXP_BIAS)
            eps_col = const.tile([P, 1], F32)
            nc.vector.memset(eps_col[:], 1e-5)

            d_all = const.tile([P, NT_GLOB], F32)
            nc.sync.dma_start(out=d_all[:], in_=deg_all[:])
            d_own = const.tile([P, NT_LOC], F32)
            nc.sync.dma_start(out=d_own[:], in_=deg_own[:])
            s_own = const.tile([P, NT_LOC], F32)
            s_all = const.tile([P, NT_GLOB], F32)
            w_col = const.tile([P, 1], F32)

            # ---- persistent big tiles ----
            KTp = [big.tile([P, N], BF16, name=f"KT{p}") for p in range(2)]
            QTp = [big.tile([P, NPC], BF16, name=f"QT{p}") for p in range(2)]
            Vt = big.tile([P, NT_GLOB * H * (DH + 1)], FP8, name="Vt")
            V4 = Vt[:].rearrange("p (k h d) -> p k h d", h=H, d=DH + 1)
            k_sb = [big.tile([P, NPC], BF16, name=f"ksb{p}") for p in range(2)]
            v_sb = big.tile([P, NT_LOC * H * (DH + 1)], FP8, name="vsb")
            V4own = v_sb[:].rearrange("p (k h d) -> p k h d", h=H, d=DH + 1)
            O_all = [big.tile([P, C], BF16, name=f"Oall{i}") for i in range(NT_LOC)]
            hi_sb = [big.tile([P, C], BF16, name=f"hi{i}") for i in range(NT_LOC)]

            kv_in = dram.tile([KVR, KVW], BF16)
            kv_out = dram.tile([NCORES * KVR, KVW], BF16)

            # ============ phase A: K/V own-shard projections + AllGather =========
            with tc.tile_pool(name="phA", bufs=1) as phA, \
                 tc.tile_pool(name="psA", bufs=1, space="PSUM") as psA:
                Wq_t = phA.tile([P, 2 * C], BF16)
                nc.sync.dma_start(out=Wq_t[:].rearrange("p (c n) -> p c n", c=2), in_=WqT[:].rearrange("(c p) n -> p c n", p=P))
                Wk_t = phA.tile([P, 2 * C], BF16)
                nc.sync.dma_start(out=Wk_t[:].rearrange("p (c n) -> p c n", c=2), in_=WkT[:].rearrange("(c p) n -> p c n", p=P))
                Wv_t = phA.tile([P, 2 * C], BF16)
                nc.sync.dma_start(out=Wv_t[:].rearrange("p (c n) -> p c n", c=2), in_=WvT[:].rearrange("(c p) n -> p c n", p=P))
                bq_t = phA.tile([P, 2], F32)
                nc.sync.dma_start(out=bq_t[:], in_=bq_pack[:])
                bk_t = phA.tile([P, 2], F32)
                nc.sync.dma_start(out=bk_t[:], in_=bk_pack[:])
                bv_t = phA.tile([P, C], F32)
                nc.sync.dma_start(out=bv_t[:], in_=bv_rep[:])
                xo = [phA.tile([P, NPC], BF16, name=f"xo{c}") for c in range(2)]
                for c in range(2):
                    nc.sync.dma_start(out=xo[c][:], in_=xT_own[c * P:(c + 1) * P, :])

                # K own
                for p in range(2):
                    for nb in range(NPC // 512):
                        kps = psA.tile([P, 512], F32, tag="qkps", bufs=2)
                        for c in range(2):
                            nc.tensor.matmul(
                                out=kps[:],
                                lhsT=Wk_t[:, c * C + p * P: c * C + (p + 1) * P],
                                rhs=xo[c][:, nb * 512:(nb + 1) * 512],
                                start=(c == 0), stop=(c == 1))
                        nc.vector.tensor_scalar(
                            out=k_sb[p][:, nb * 512:(nb + 1) * 512], in0=kps[:],
                            scalar1=bk_t[:, p:p + 1], scalar2=None, op0=OP.add)
                # V own (fp8, ones col for denominators)
                nc.vector.memset(V4own[:, :, :, DH:DH + 1], 1.0)
                for ntl in range(NT_LOC):
                    vps = psA.tile([P, C], F32, tag="vps", bufs=2)
                    for c in range(2):
                        nc.tensor.matmul(
                            out=vps[:],
                            lhsT=xo[c][:, ntl * P:(ntl + 1) * P],
                            rhs=Wv_t[:, c * C:(c + 1) * C],
                            start=(c == 0), stop=(c == 1))
                    nc.vector.tensor_tensor(
                        out=V4own[:, ntl, :, 0:DH],
                        in0=vps[:].rearrange("p (h d) -> p h d", d=DH),
                        in1=bv_t[:].rearrange("p (h d) -> p h d", d=DH),
                        op=OP.add)
                # stage K/V and launch the AllGather
                for p in range(2):
                    nc.scalar.dma_start(out=kv_in[p * P:(p + 1) * P, 0:NPC], in_=k_sb[p][:])
                nc.scalar.dma_start(out=kv_in[2 * P:3 * P, :],
                                    in_=v_sb[:].bitcast(BF16))
                nc.gpsimd.collective_compute(
                    "AllGather", OP.bypass,
                    replica_groups=[list(range(NCORES))],
                    ins=[kv_in[:].opt()], outs=[kv_out[:].opt()])

                # sigmoid(alpha) -> w column (while collective runs)
                al_t = phA.tile([1, 1], F32)
                nc.scalar.dma_start(out=al_t[:], in_=alpha11[:])
                wsig = phA.tile([1, 1], F32)
                nc.scalar.activation(out=wsig[:], in_=al_t[:], func=AF.Sigmoid)
                wrep_ps = psA.tile([P, 1], F32, tag="wrep")
                nc.tensor.matmul(out=wrep_ps[:], lhsT=ones_row_t[:], rhs=wsig[:],
                                 start=True, stop=True)
                nc.vector.tensor_copy(out=w_col[:], in_=wrep_ps[:])

                # Q projection
                for p in range(2):
                    for nb in range(NPC // 512):
                        qps = psA.tile([P, 512], F32, tag="qkps", bufs=2)
                        for c in range(2):
                            nc.tensor.matmul(
                                out=qps[:],
                                lhsT=Wq_t[:, c * C + p * P: c * C + (p + 1) * P],
                                rhs=xo[c][:, nb * 512:(nb + 1) * 512],
                                start=(c == 0), stop=(c == 1))
                        nc.vector.tensor_scalar(
                            out=QTp[p][:, nb * 512:(nb + 1) * 512], in0=qps[:],
                            scalar1=bq_t[:, p:p + 1], scalar2=None, op0=OP.add)

                # rsqrt scale tables from host degrees (guarded like reference)
                for (src_ap, dst, w_) in ((d_all[:], s_all, NT_GLOB),
                                          (d_own[:], s_own, NT_LOC)):
                    m_t = phA.tile([P, w_], F32, tag=f"m{w_}")
                    nc.vector.tensor_scalar(out=m_t[:], in0=src_ap, scalar1=1.0,
                                            scalar2=None, op0=OP.min)
                    t1 = phA.tile([P, w_], F32, tag=f"t1{w_}")
                    nc.vector.tensor_scalar(out=t1[:], in0=src_ap, scalar1=1.0,
                                            scalar2=None, op0=OP.add)
                    nc.vector.tensor_tensor(out=t1[:], in0=t1[:], in1=m_t[:],
                                            op=OP.subtract)
                    nc.scalar.activation(out=t1[:], in_=t1[:], func=AF.Sqrt)
                    nc.vector.reciprocal(out=t1[:], in_=t1[:])
                    nc.vector.tensor_tensor(out=dst[:], in0=t1[:], in1=m_t[:],
                                            op=OP.mult)

            # ============ phase B: y table + K/V gather-in ========================
            # y = x * rsqrt(d): input tiles on sync, output tiles on gpsimd (the
            # same queue the scatter gathers use, so writes order before reads).
            with tc.tile_pool(name="phB", bufs=1) as phB:
                for g in range(NT_GLOB):
                    xt = phB.tile([P, C], BF16, tag="xt", bufs=4)
                    nc.sync.dma_start(out=xt[:], in_=x_full[g * P:(g + 1) * P, :])
                    yt = phB.tile([P, C], BF16, tag="yt", bufs=4)
                    nc.vector.tensor_scalar(out=yt[:], in0=xt[:],
                                            scalar1=s_all[:, g:g + 1],
                                            scalar2=None, op0=OP.mult)
                    nc.gpsimd.dma_start(out=y_scr[g * P:(g + 1) * P, :], in_=yt[:])

            # K/V loads (sync queue; waits on the collective)
            for k in range(NCORES):
                for p in range(2):
                    nc.sync.dma_start(
                        out=KTp[p][:, k * NPC:(k + 1) * NPC],
                        in_=kv_out[k * KVR + p * P: k * KVR + (p + 1) * P, 0:NPC])
            for k in range(NCORES):
                nc.sync.dma_start(
                    out=Vt[:, k * NT_LOC * H * (DH + 1):(k + 1) * NT_LOC * H * (DH + 1)].bitcast(BF16),
                    in_=kv_out[k * KVR + 2 * P: (k + 1) * KVR, :])

            # ============ phase C: attention + interleaved GCN scatter ===========
            with tc.tile_pool(name="phC", bufs=1) as phC, \
                 tc.tile_pool(name="psC", bufs=1, space="PSUM") as psC:

                scat_jobs = [(t, i) for t in range(NT_LOC) for i in range(TPT)]
                n_jobs = len(scat_jobs)
                n_steps = H * NT_GLOB
                emitted = 0
                hips_cur = {}

                def emit_scatter_jobs(upto):
                    nonlocal emitted
                    while emitted < min(upto, n_jobs):
                        t, i = scat_jobs[emitted]
                        j = t * TPT + i
                        if i == 0:
                            hips_cur[t] = psC.tile([P, C], F32, tag="hips", bufs=2, name=f"hips{t}")
                        yg = phC.tile([P, C], BF16, tag="yg", bufs=6)
                        nc.gpsimd.indirect_dma_start(
                            out=yg[:], out_offset=None, in_=y_scr[:],
                            in_offset=bass.IndirectOffsetOnAxis(
                                ap=row_t[:, j:j + 1], axis=0))
                        oh = phC.tile([P, P], BF16, tag="oh2", bufs=3)
                        nc.vector.tensor_scalar(
                            out=oh[:], in0=iota_b[:], scalar1=colf_t[:, j:j + 1],
                            scalar2=mult_t[:, j:j + 1], op0=OP.is_equal, op1=OP.mult)
                        nc.tensor.matmul(out=hips_cur[t][:], lhsT=oh[:], rhs=yg[:],
                                         start=(i == 0), stop=(i == TPT - 1))
                        if i == TPT - 1:
                            nc.vector.tensor_scalar(out=hi_sb[t][:], in0=hips_cur[t][:],
                                                    scalar1=s_own[:, t:t + 1],
                                                    scalar2=None, op0=OP.mult)
                        emitted += 1

                for h in range(H):
                    p, hh = h // 2, h % 2
                    po = hh * DH
                    Ops = [psC.tile([P, 4 * (DH + 1)], F32, tag=f"O{i}", bufs=1, name=f"O{i}h{h}")
                           for i in range(2)]
                    for kt in range(NT_GLOB):
                        sps = psC.tile([P, NPC], F32, tag="sps", bufs=2)
                        for qh in range(2):
                            nc.tensor.matmul(
                                out=sps[:, qh * 512:(qh + 1) * 512],
                                lhsT=KTp[p][po:po + DH, kt * P:(kt + 1) * P],
                                rhs=QTp[p][po:po + DH, qh * 512:(qh + 1) * 512],
                                start=True, stop=True)
                        if kt % 2 == 0:
                            et2 = phC.tile([P, 2 * NPC], FP8E5, tag="expT", bufs=2)
                            e3 = et2[:].rearrange("p (z n) -> p z n", z=2)
                        nc.scalar.activation(out=e3[:, kt % 2, :], in_=sps[:],
                                             func=AF.Exp, bias=expb_col[:, 0:1],
                                             scale=1.0 / np.sqrt(DH))
                        if kt % 2 == 1:
                            for qt in range(NT_LOC):
                                nc.tensor.matmul(
                                    out=Ops[qt // 4][:, (qt % 4) * (DH + 1):(qt % 4 + 1) * (DH + 1)],
                                    lhsT=e3[:, :, qt * P:(qt + 1) * P],
                                    rhs=V4[:, kt - 1:kt + 1, h, :],
                                    start=(kt == 1), stop=(kt == NT_GLOB - 1),
                                    perf_mode=DR)
                        step = h * NT_GLOB + kt + 1
                        emit_scatter_jobs(n_jobs * step // n_steps)
                    for qt in range(NT_LOC):
                        sl0 = (qt % 4) * (DH + 1)
                        den = phC.tile([P, 1], F32, tag="den", bufs=2)
                        nc.vector.reciprocal(out=den[:],
                                             in_=Ops[qt // 4][:, sl0 + DH:sl0 + DH + 1])
                        nc.vector.tensor_scalar(
                            out=O_all[qt][:, h * DH:(h + 1) * DH],
                            in0=Ops[qt // 4][:, sl0:sl0 + DH],
                            scalar1=den[:, 0:1], scalar2=None, op0=OP.mult)

            # ============ phase D: out_proj, LN, combine, fc (stage-major) =======
            with tc.tile_pool(name="phD", bufs=1) as phD, \
                 tc.tile_pool(name="psD", bufs=1, space="PSUM") as psD:
                Wop_t = phD.tile([P, 2 * C], BF16)
                nc.sync.dma_start(out=Wop_t[:].rearrange("p (c n) -> p c n", c=2), in_=WopT[:].rearrange("(c p) n -> p c n", p=P))
                Wl_t = phD.tile([P, 2 * C], BF16)
                nc.sync.dma_start(out=Wl_t[:].rearrange("p (c n) -> p c n", c=2), in_=Wl[:].rearrange("(c p) n -> p c n", p=P))
                fc_t = phD.tile([P, 2 * OUTC], BF16)
                nc.sync.dma_start(out=fc_t[:].rearrange("p (c n) -> p c n", c=2), in_=fcT[:].rearrange("(c p) n -> p c n", p=P))
                opb_t = phD.tile([P, C], F32)
                nc.sync.dma_start(out=opb_t[:], in_=opb_rep[:])
                g_t = phD.tile([P, C], F32)
                nc.sync.dma_start(out=g_t[:], in_=g_rep[:])
                b_t = phD.tile([P, C], F32)
                nc.sync.dma_start(out=b_t[:], in_=b_rep[:])
                fcb_t = phD.tile([P, OUTC], F32)
                nc.sync.dma_start(out=fcb_t[:], in_=fcb_rep[:])
                xo_t = [phD.tile([P, C], F32, name=f"xot{q}") for q in range(NT_LOC)]
                for qt in range(NT_LOC):
                    nc.sync.dma_start(out=xo_t[qt][:], in_=x_own[qt * P:(qt + 1) * P, :])

                def transpose_2chunks(src_ap, dst):
                    for c in range(2):
                        tp = psD.tile([P, P], BF16, tag="tp", bufs=2)
                        nc.tensor.transpose(out=tp[:], in_=src_ap[:, c * P:(c + 1) * P],
                                            identity=ident_b[:])
                        nc.vector.tensor_copy(out=dst[:, c * P:(c + 1) * P], in_=tp[:])

                OT = [phD.tile([P, C], BF16, name=f"OTq{q}") for q in range(NT_LOC)]
                hiT = [phD.tile([P, C], BF16, name=f"hiTq{q}") for q in range(NT_LOC)]
                v_t = [phD.tile([P, C], F32, name=f"vtq{q}") for q in range(NT_LOC)]
                vn = [phD.tile([P, C], F32, name=f"vnq{q}") for q in range(NT_LOC)]
                comb = [phD.tile([P, C], BF16, name=f"combq{q}") for q in range(NT_LOC)]
                cT = [phD.tile([P, C], BF16, name=f"cTq{q}") for q in range(NT_LOC)]

                for qt in range(NT_LOC):
                    transpose_2chunks(O_all[qt][:], OT[qt])
                    transpose_2chunks(hi_sb[qt][:], hiT[qt])
                for qt in range(NT_LOC):
                    aps = psD.tile([P, C], F32, tag="aps", bufs=2)
                    for c in range(2):
                        nc.tensor.matmul(out=aps[:], lhsT=OT[qt][:, c * P:(c + 1) * P],
                                         rhs=Wop_t[:, c * C:(c + 1) * C],
                                         start=(c == 0), stop=(c == 1))
                    nc.vector.tensor_tensor(out=v_t[qt][:], in0=aps[:], in1=opb_t[:], op=OP.add)
                    nc.vector.tensor_tensor(out=v_t[qt][:], in0=v_t[qt][:], in1=xo_t[qt][:], op=OP.add)
                for qt in range(NT_LOC):
                    vcop = phD.tile([P, C], F32, tag="vcop", bufs=2)
                    msum = phD.tile([P, 1], F32, tag="msum", bufs=4)
                    nc.scalar.activation(out=vcop[:], in_=v_t[qt][:], func=AF.Copy,
                                         accum_out=msum[:])
                    mean = phD.tile([P, 1], F32, tag="mean", bufs=4)
                    nc.vector.tensor_scalar(out=mean[:], in0=msum[:], scalar1=1.0 / C,
                                            scalar2=None, op0=OP.mult)
                    nc.vector.tensor_scalar(out=v_t[qt][:], in0=v_t[qt][:],
                                            scalar1=mean[:, 0:1],
                                            scalar2=None, op0=OP.subtract)
                    sq = phD.tile([P, C], F32, tag="sq", bufs=2)
                    ssum = phD.tile([P, 1], F32, tag="ssum", bufs=4)
                    nc.scalar.activation(out=sq[:], in_=v_t[qt][:], func=AF.Square,
                                         accum_out=ssum[:])
                    sstd = phD.tile([P, 1], F32, tag="sstd", bufs=4)
                    nc.scalar.activation(out=sstd[:], in_=ssum[:], func=AF.Sqrt,
                                         bias=eps_col[:, 0:1], scale=1.0 / C)
                    rstd = phD.tile([P, 1], F32, tag="rstd", bufs=4)
                    nc.vector.reciprocal(out=rstd[:], in_=sstd[:])
                    nc.scalar.activation(out=vn[qt][:], in_=v_t[qt][:], func=AF.Copy,
                                         scale=rstd[:, 0:1])
                    nc.vector.tensor_tensor(out=vn[qt][:], in0=vn[qt][:], in1=g_t[:], op=OP.mult)
                    nc.vector.tensor_tensor(out=vn[qt][:], in0=vn[qt][:], in1=b_t[:], op=OP.add)
                for qt in range(NT_LOC):
                    lps = psD.tile([P, C], F32, tag="lps", bufs=2)
                    for c in range(2):
                        nc.tensor.matmul(out=lps[:], lhsT=hiT[qt][:, c * P:(c + 1) * P],
                                         rhs=Wl_t[:, c * C:(c + 1) * C],
                                         start=(c == 0), stop=(c == 1))
                    df = phD.tile([P, C], F32, tag="df", bufs=3)
                    nc.vector.tensor_tensor(out=df[:], in0=lps[:], in1=vn[qt][:],
                                            op=OP.subtract)
                    nc.vector.tensor_scalar(out=df[:], in0=df[:], scalar1=w_col[:, 0:1],
                                            scalar2=None, op0=OP.mult)
                    nc.vector.tensor_tensor(out=comb[qt][:], in0=df[:], in1=vn[qt][:], op=OP.add)
                for qt in range(NT_LOC):
                    transpose_2chunks(comb[qt][:], cT[qt])
                    fps = psD.tile([P, OUTC], F32, tag="fps", bufs=2)
                    for c in range(2):
                        nc.tensor.matmul(out=fps[:], lhsT=cT[qt][:, c * P:(c + 1) * P],
                                         rhs=fc_t[:, c * OUTC:(c + 1) * OUTC],
                                         start=(c == 0), stop=(c == 1))
                    o_t = phD.tile([P, OUTC], F32, tag="ot", bufs=3)
                    nc.vector.tensor_tensor(out=o_t[:], in0=fps[:], in1=fcb_t[:], op=OP.add)
                    nc.sync.dma_start(out=out[qt * P:(qt + 1) * P, :], in_=o_t[:])
    nc.finalize()
    return nc


def _prep_edges(adj):
    """Dedup (row, col) pairs, bucket by destination node-tile, pad segments.

    Returns per-core (col_adj, row_idx, mult) [P, TE] arrays, TPT, and the
    per-node degree histogram (with duplicate multiplicity).
    """
    row = np.asarray(adj[0], dtype=np.int64)
    col = np.asarray(adj[1], dtype=np.int64)
    deg = np.bincount(col, minlength=N).astype(np.float32)
    key = col * N + row
    ukey, counts = np.unique(key, return_counts=True)
    ucol = (ukey // N).astype(np.int64)
    urow = (ukey % N).astype(np.int64)
    tid = ucol // P
    seg_counts = np.bincount(tid, minlength=NT_GLOB)
    S = int(np.ceil(max(seg_counts.max(), 1) / P) * P)
    TPT = S // P
    col_pad = np.full((NT_GLOB, S), -1, dtype=np.int32)
    row_pad = np.zeros((NT_GLOB, S), dtype=np.int32)
    mul_pad = np.zeros((NT_GLOB, S), dtype=np.float32)
    start = 0
    for g in range(NT_GLOB):
        cnt = int(seg_counts[g])
        col_pad[g, :cnt] = (ucol[start:start + cnt] - g * P).astype(np.int32)
        row_pad[g, :cnt] = urow[start:start + cnt].astype(np.int32)
        mul_pad[g, :cnt] = counts[start:start + cnt].astype(np.float32)
        start += cnt
    col_pad = col_pad.reshape(NT_GLOB, TPT, P)
    row_pad = row_pad.reshape(NT_GLOB, TPT, P)
    mul_pad = mul_pad.reshape(NT_GLOB, TPT, P)
    per_core = []
    for k in range(NCORES):
        ca = col_pad[NT_LOC * k:NT_LOC * (k + 1)].reshape(NT_LOC * TPT, P).T
        ri = row_pad[NT_LOC * k:NT_LOC * (k + 1)].reshape(NT_LOC * TPT, P).T
        mu = mul_pad[NT_LOC * k:NT_LOC * (k + 1)].reshape(NT_LOC * TPT, P).T
        per_core.append((np.ascontiguousarray(ca), np.ascontiguousarray(ri),
                         np.ascontiguousarray(mu)))
    return per_core, TPT, deg


def kernel(x, adj, weight_local, in_proj_w, in_proj_b, out_proj_w, out_proj_b,
           ln_g, ln_b, alpha, fc_w, fc_b):
    global LAST_RESULTS
    BF = ml_dtypes.bfloat16
    x = np.ascontiguousarray(np.asarray(x, dtype=np.float32))
    per_core_edges, TPT, deg = _prep_edges(np.asarray(adj))
    deg_pack = np.ascontiguousarray(deg.reshape(NT_GLOB, P).T)  # [P, 64]

    xT_b = np.ascontiguousarray(x.T).astype(BF)
    common = dict(
        x_full=x.astype(BF),
        deg_all=deg_pack,
        WqT=np.ascontiguousarray(np.asarray(in_proj_w)[0:C].T.astype(np.float32)).astype(BF),
        WkT=np.ascontiguousarray(np.asarray(in_proj_w)[C:2 * C].T.astype(np.float32)).astype(BF),
        WvT=np.ascontiguousarray(np.asarray(in_proj_w)[2 * C:3 * C].T.astype(np.float32)).astype(BF),
        WopT=np.ascontiguousarray(np.asarray(out_proj_w).T.astype(np.float32)).astype(BF),
        Wl=np.ascontiguousarray(np.asarray(weight_local, dtype=np.float32)).astype(BF),
        fcT=np.ascontiguousarray(np.asarray(fc_w).T.astype(np.float32)).astype(BF),
        bq_pack=np.ascontiguousarray(np.asarray(in_proj_b)[0:C].astype(np.float32).reshape(2, P).T),
        bk_pack=np.ascontiguousarray(np.asarray(in_proj_b)[C:2 * C].astype(np.float32).reshape(2, P).T),
        bv_rep=np.tile(np.asarray(in_proj_b)[2 * C:3 * C].astype(np.float32), (P, 1)),
        opb_rep=np.tile(np.asarray(out_proj_b, dtype=np.float32), (P, 1)),
        g_rep=np.tile(np.asarray(ln_g, dtype=np.float32), (P, 1)),
        b_rep=np.tile(np.asarray(ln_b, dtype=np.float32), (P, 1)),
        fcb_rep=np.tile(np.asarray(fc_b, dtype=np.float32), (P, 1)),
        alpha11=np.asarray(alpha, dtype=np.float32).reshape(1, 1),
        iota_in=np.tile(np.arange(P, dtype=np.float32), (P, 1)),
        ident_in=np.eye(P, dtype=np.float32),
        ones_row_in=np.ones((1, P), dtype=np.float32),
    )
    in_maps = []
    for k in range(NCORES):
        ca, ri, mu = per_core_edges[k]
        m = dict(common)
        m['xT_own'] = np.ascontiguousarray(xT_b[:, k * NPC:(k + 1) * NPC])
        m['x_own'] = np.ascontiguousarray(x[k * NPC:(k + 1) * NPC, :])
        m['deg_own'] = np.ascontiguousarray(deg_pack[:, k * NT_LOC:(k + 1) * NT_LOC])
        m['col_adj'] = ca
        m['row_idx'] = ri
        m['mult_in'] = mu
        in_maps.append(m)

    nc = _build(TPT)
    res = run_bass_kernel_spmd(nc, in_maps, core_ids=list(range(NCORES)))
    LAST_RESULTS = res
    return np.concatenate([res.results[k]['out'] for k in range(NCORES)], axis=0)
